# revision 35
# baseline (speedup 1.0000x reference)
import sys
sys.path.insert(0, "/opt/trn_rl_repo")
"""Dimer GNN on 8 TRN2 cores — v4 (fused single-phase).

Edge-parallel sharding by owner core (dir 0: owner=dst updates y1, gathers
y0; dir 1: owner=src updates y0, gathers y1). ~50k edges per (core, dir).

Slot layout per direction (geometry COMMON across cores): tiles = columns
[P, T]; tiles are dst-block-pure (49 local blocks of 128 nodes); within a
block, edges are packed into 64-slot HALF-COLUMN strips that are bucket-pure
(strip count per (block, bucket) = max over cores -> common static geometry).
Tile t therefore has 1-2 static TP segments (partition ranges 0:64 / 64:128),
each with a static bucket.

Per direction-layer, per 4-tile quad (fused pipeline):
  atoms: l=0 from host-pregathered slab; l>=1 per-tile [128,1] indirect DMA
  row-gather from the AllGathered bf16 y_tab [N, 32].
  DVE rep-copy -> [128,(t,4rep,32j)]; 4 PE transposes -> psum atomsT_rep
  bf16 [128(d,j), 512]; DVE multiply with streamed pwrepT (u^d, channel-
  major, dead slots 0) -> xpT; per tile 1-2 MMs (lhsT=xpT segment, rhs=
  Wbflat[l,dir,bucket]) -> m [128e, 32k] f32 psum; ACT Silu -> act quad
  bf16; per tile: one-hot (DVE is_equal vs dloc) + scatter matmul
  accumulating into the block's [32,128] psum; block end: DVE add into
  channel-major yT [32, 6272] f32.
Layer end per side: PE transposes of yT -> bf16 rows -> AllGather y_tab.
Readout: chunked PE dot + ACT Silu + DVE reduce + AllReduce.
"""
import numpy as np

DIM = 32
STEP = 0.25   # gaussian sigma (fixed by the model)
N_MU = 21
N_SCAL = N_MU + 1
N_LAYERS = 3
NB = 10
BW = 5.0 / NB  # polynomial bucket width
P = 128
D = 4
HS = 64  # strip size (half column)


def poly_coeffs():
    mu = np.linspace(0.0, 5.0, N_MU)
    C = np.zeros((NB, D, N_SCAL))
    for b in range(NB):
        rr = np.linspace(b * BW, (b + 1) * BW, 257)
        u = (rr - b * BW) / BW * 2.0 - 1.0
        V = np.stack([u ** d for d in range(D)], 1)
        G = np.exp(-0.5 * ((rr[:, None] - mu[None, :]) / STEP) ** 2)
        G = np.concatenate([G, np.ones((len(rr), 1))], 1)
        C[b], *_ = np.linalg.lstsq(V, G, rcond=None)
    return C  # [NB, D, N_SCAL]


def _ceil(a, b):
    return -(-a // b)


def shard_direction(owner, other, r, n_cores, npc, G):
    """Fused layout. Returns per-core arrays + static geometry."""
    core_of = owner // npc
    bucket = np.clip((r / BW).astype(np.int64), 0, NB - 1)
    n_blocks = _ceil(npc, P)

    idx_by_core = [np.nonzero(core_of == c)[0] for c in range(n_cores)]
    loc_all = owner - core_of * npc
    blk_all = loc_all // P

    # common geometry: strips per (block, bucket) = max over cores
    nst = np.zeros((n_blocks, NB), np.int64)
    for c in range(n_cores):
        idx = idx_by_core[c]
        for kb in range(n_blocks):
            sel = idx[blk_all[idx] == kb]
            cnt = np.bincount(bucket[sel], minlength=NB)
            nst[kb] = np.maximum(nst[kb], _ceil(cnt, HS))
    S_kb = nst.sum(1)                       # strips per block
    tiles_kb = _ceil(S_kb, 2)               # 2 strips per tile
    Toff = np.concatenate([[0], np.cumsum(tiles_kb)]).astype(np.int64)
    T_live = int(Toff[-1])
    T = _ceil(T_live, G) * G

    # static segments: for each tile, list of (seg_lo_strip(0/1), bucket)
    # strip sigma of block kb: tile Toff[kb] + sigma//2, half sigma%2
    seg_bucket = np.full((T, 2), -1, np.int64)  # -1 = dead segment
    strip_pos = {}  # (kb, b) -> list of (tile, half) strips
    for kb in range(n_blocks):
        sigma = 0
        for b in range(NB):
            lst = []
            for _ in range(int(nst[kb, b])):
                t = int(Toff[kb] + sigma // 2)
                h = sigma % 2
                seg_bucket[t, h] = b
                lst.append((t, h))
                sigma += 1
            strip_pos[(kb, b)] = lst

    cores = []
    for c in range(n_cores):
        idx = idx_by_core[c]
        uA = np.zeros((P, T))
        vA = np.zeros((P, T))
        gidx = np.zeros((P, T), np.int32)
        dloc = np.full((P, T), 255.0)
        for kb in range(n_blocks):
            bidx = idx[blk_all[idx] == kb]
            bb = bucket[bidx]
            for b in range(NB):
                sel = bidx[bb == b]
                if len(sel) == 0:
                    continue
                sel = sel[np.argsort(other[sel], kind="stable")]
                strips = strip_pos[(kb, b)]
                for si, (t, h) in enumerate(strips):
                    seg = sel[si * HS:(si + 1) * HS]
                    if len(seg) == 0:
                        break
                    pr = h * HS + np.arange(len(seg))
                    uA[pr, t] = (r[seg] - b * BW) / BW * 2.0 - 1.0
                    vA[pr, t] = 1.0
                    gidx[pr, t] = other[seg]
                    dloc[pr, t] = (loc_all[seg] % P).astype(np.float64)
        cores.append(dict(uA=uA, vA=vA, gidx=gidx, dloc=dloc))

    meta = dict(T=T, T_live=T_live, Toff=Toff, seg_bucket=seg_bucket,
                n_blocks=n_blocks, tiles_kb=tiles_kb)
    return cores, meta


def prepare(inputs, n_cores=8, G=32):
    import ml_dtypes
    BF16 = ml_dtypes.bfloat16

    z0 = np.asarray(inputs["z0"]).astype(np.int64)
    z1 = np.asarray(inputs["z1"]).astype(np.int64)
    src = np.asarray(inputs["src"]).astype(np.int64)
    dst = np.asarray(inputs["dst"]).astype(np.int64)
    r = np.asarray(inputs["r"], np.float64)
    emb_w = np.asarray(inputs["emb_w"], np.float32)
    emb_b = np.asarray(inputs["emb_b"], np.float32)
    w_s2d = np.asarray(inputs["w_s2d"], np.float32)
    w_d2s = np.asarray(inputs["w_d2s"], np.float32)
    ro_w = np.asarray(inputs["ro_w"], np.float32)
    ro_b = np.asarray(inputs["ro_b"], np.float32)
    N0, N1 = len(z0), len(z1)
    npc0, npc1 = N0 // n_cores, N1 // n_cores

    C = poly_coeffs()
    scale = 1.0 / np.sqrt(N_SCAL * DIM) / np.sqrt(N0 + N1)
    wb = np.zeros((N_LAYERS, 2, NB, D * DIM, DIM), np.float32)
    for l in range(N_LAYERS):
        wb[l, 0] = np.einsum("bdi,ijk->bdjk", C, w_s2d[l]).reshape(NB, D * DIM, DIM) * scale
        wb[l, 1] = np.einsum("bdi,ijk->bdjk", C, w_d2s[l]).reshape(NB, D * DIM, DIM) * scale
    wb_sb = np.ascontiguousarray(
        wb.transpose(3, 0, 1, 2, 4).reshape(D * DIM, N_LAYERS * 2 * NB * DIM)
    ).astype(BF16)

    emb_eff = (emb_w + emb_b[None, :]).astype(np.float32)
    emb_b16 = emb_eff.astype(BF16)

    s2d_cores, s2d_meta = shard_direction(dst, src, r, n_cores, npc1, G)
    d2s_cores, d2s_meta = shard_direction(src, dst, r, n_cores, npc0, G)
    metas = {0: s2d_meta, 1: d2s_meta}
    z_of_side = {0: z0, 1: z1}
    gside_of_dir = {0: 0, 1: 1}
    npc_of_side = {0: npc0, 1: npc1}

    per_core = []
    for c in range(n_cores):
        dd = {}
        for d, cores in ((0, s2d_cores), (1, d2s_cores)):
            cd = cores[c]
            T = metas[d]["T"]
            u, v = cd["uA"], cd["vA"]
            pows = np.stack([v * (u ** dg) for dg in range(D)], 0)
            pw = np.repeat(pows, DIM, axis=0)  # [128 (d,j), P e, T]
            # edge-major: [P e, T*(d,j)] for the fused scale-replicate copy
            pw_em = np.ascontiguousarray(
                pw.transpose(1, 2, 0).reshape(P, T * D * DIM)).astype(BF16)
            zg = z_of_side[gside_of_dir[d]]
            # l=0 slab: pre-transposed, pre-scaled lhsT  [128 (d,j), T*P]
            atT = emb_eff[zg[cd["gidx"]]]            # [P e, T, DIM j] f32
            atT = atT.transpose(2, 1, 0)             # [DIM j, T, P e]
            atT_rep = np.tile(atT, (D, 1, 1)).reshape(D * DIM, T * P)
            pw_t = pw.transpose(0, 2, 1).reshape(D * DIM, T * P)
            sfx = f"_{d}"
            dd["pwem" + sfx] = pw_em
            dd["xpT0" + sfx] = (pw_t * atT_rep).astype(BF16)
            dd["gidx" + sfx] = np.ascontiguousarray(cd["gidx"])
            dd["dloc" + sfx] = np.ascontiguousarray(cd["dloc"].astype(BF16))
        for s in (0, 1):
            npc = npc_of_side[s]
            nblk = _ceil(npc, P)
            zz = np.zeros(nblk * P, np.int64)
            zz[:npc] = z_of_side[s][c * npc:(c + 1) * npc]
            yT0 = emb_eff[zz].T.copy()
            yT0[:, npc:] = 0.0
            dd[f"yT0_{s}"] = np.ascontiguousarray(yT0)  # [32, nblk*P] f32
            dd[f"ytab0_{s}"] = emb_b16[z_of_side[s]]     # [N, 32] bf16
        dd["wb_sb"] = wb_sb
        dd["ro_w"] = ro_w
        per_core.append(dd)

    meta = dict(m0=s2d_meta, m1=d2s_meta, n_cores=n_cores, G=G,
                npc0=npc0, npc1=npc1, ro_b=float(ro_b[0]),
                N_of_side={0: N0, 1: N1})
    return per_core, meta


# ======================== bass program ========================
from contextlib import ExitStack

from concourse import bass, mybir
from concourse import bacc
import concourse.tile as tile
from concourse.masks import make_identity

FP = mybir.dt.float32
BF = mybir.dt.bfloat16
I32 = mybir.dt.int32
AF = mybir.ActivationFunctionType
ALU = mybir.AluOpType
QUAD = 8


def build_program(meta, shapes, n_layers=3, sim_mode=False):
    n_cores = meta["n_cores"]
    G = meta["G"]
    npc = {0: meta["npc0"], 1: meta["npc1"]}
    N_side = meta["N_of_side"]
    metas = {0: meta["m0"], 1: meta["m1"]}
    side_of_dir = {0: 1, 1: 0}
    gside_of_dir = {0: 0, 1: 1}

    nc = bacc.Bacc("TRN2", target_bir_lowering=False, debug=False,
                   num_devices=1 if sim_mode else n_cores)

    def emit_collective(kind, op, ins, outs, replica_groups):
        if sim_mode:
            # single-core timing model: stand in a local DMA of the same
            # output footprint (collective itself modeled separately)
            nc.gpsimd.dma_start(out=outs[0], in_=outs[0])
        else:
            nc.gpsimd.collective_compute(
                kind, op, ins=ins, outs=outs, replica_groups=replica_groups)

    ins = {}
    for name, shp in shapes.items():
        if name.startswith("gidx"):
            dt = I32
        elif name.startswith(("yT0", "ro_w")):
            dt = FP
        else:
            dt = BF
        ins[name] = nc.dram_tensor(name, list(shp), dt, kind="ExternalInput")

    out_t = nc.dram_tensor("out", [1, 1], FP, kind="ExternalOutput")

    cc_in = {s: nc.dram_tensor(f"cc_in_{s}", [npc[s], DIM], BF)
             for s in (0, 1)}
    y_tab = {}
    for s in (0, 1):
        for l in (1, 2):
            y_tab[(s, l)] = nc.dram_tensor(
                f"y_tab_{s}_{l}", [N_side[s], DIM], BF, addr_space="Shared")
    ar_in = nc.dram_tensor("ar_in", [1, 1], FP)
    ar_out = nc.dram_tensor("ar_out", [1, 1], FP, addr_space="Shared")

    replica_groups = [list(range(n_cores))]
    WBCOLS = n_layers * 2 * NB * DIM
    nblk_of = {s: _ceil(npc[s], P) for s in (0, 1)}

    with tile.TileContext(nc) as tc, ExitStack() as ctx:
        const = ctx.enter_context(tc.tile_pool(name="const", bufs=1))
        identity = const.tile([P, P], FP)
        make_identity(nc, identity[:])
        ident_b = const.tile([P, P], BF)
        nc.vector.tensor_copy(out=ident_b[:], in_=identity[:])
        iota_i = const.tile([P, P], I32)
        nc.gpsimd.iota(iota_i[:], [[1, P]], base=0, channel_multiplier=0)
        iota_b = const.tile([P, P], BF)
        nc.vector.tensor_copy(out=iota_b[:], in_=iota_i[:])
        iota4 = const.tile([P, QUAD * P], BF)
        for _q in range(QUAD):
            nc.vector.tensor_copy(out=iota4[:, _q * P:(_q + 1) * P],
                                  in_=iota_b[:])
        ro_w_sb = const.tile([DIM, 1], FP)
        nc.sync.dma_start(ro_w_sb[:], ins["ro_w"][:])
        wb_sb = const.tile([P, WBCOLS], BF)
        nc.sync.dma_start(wb_sb[:], ins["wb_sb"][:])

        def wb_ap(l, d, b):
            off = ((l * 2 + d) * NB + b) * DIM
            return wb_sb[:, off:off + DIM]

        emeta = ctx.enter_context(tc.tile_pool(name="emeta", bufs=1))
        gidx_sb, dloc_sb = {}, {}
        for d in (0, 1):
            T = metas[d]["T"]
            gidx_sb[d] = emeta.tile([P, T], I32, name=f"gidx{d}", tag=f"gidx{d}")
            nc.sync.dma_start(gidx_sb[d][:], ins[f"gidx_{d}"][:])
            dloc_sb[d] = emeta.tile([P, T], BF, name=f"dloc{d}", tag=f"dloc{d}")
            nc.sync.dma_start(dloc_sb[d][:], ins[f"dloc_{d}"][:])

        yT = {}
        for s in (0, 1):
            yT[s] = const.tile([DIM, nblk_of[s] * P], FP, name=f"yT{s}", tag=f"yT{s}")
            nc.sync.dma_start(yT[s][:], ins[f"yT0_{s}"][:])

        # ---------------- fused direction-layer ----------------
        def emit_dirlayer(l, d):
            m = metas[d]
            T, T_live = m["T"], m["T_live"]
            Toff = m["Toff"]
            seg_bucket = m["seg_bucket"]
            n_blocks = m["n_blocks"]
            s = side_of_dir[d]
            gs = gside_of_dir[d]
            # block of tile, and first/last live tile per block
            blk_of = np.searchsorted(Toff, np.arange(T), side="right") - 1
            with ExitStack() as actx:
                pA = actx.enter_context(tc.tile_pool(name=f"pA{l}{d}", bufs=3))
                pX = actx.enter_context(tc.tile_pool(name=f"pX{l}{d}", bufs=4))
                pO = actx.enter_context(tc.tile_pool(name=f"pO{l}{d}", bufs=4))
                psT = actx.enter_context(
                    tc.tile_pool(name=f"psT{l}{d}", bufs=3, space="PSUM"))
                psM = actx.enter_context(
                    tc.tile_pool(name=f"psM{l}{d}", bufs=3, space="PSUM"))
                psB = actx.enter_context(
                    tc.tile_pool(name=f"psB{l}{d}", bufs=2, space="PSUM"))
                yacc = None
                nG = T // G
                for g in range(nG):
                    t0 = g * G
                    nlive = max(0, min(G, T_live - t0))
                    if nlive == 0:
                        continue
                    if l == 0:
                        xq = pA.tile([P, G * P], BF, tag="pw")
                        nc.sync.dma_start(
                            xq[:], ins[f"xpT0_{d}"][:, t0 * P:(t0 + G) * P])
                    else:
                        atoms = pA.tile([P, G * DIM], BF, tag="atoms")
                        tab = y_tab[(gs, l)]
                        nc.gpsimd.indirect_dma_start(
                            out=atoms[:].rearrange(
                                "p (t j) -> p t j", j=DIM),
                            out_offset=None, in_=tab[:],
                            in_offset=bass.IndirectOffsetOnAxis(
                                ap=gidx_sb[d][:, t0:t0 + G], axis=0))
                        pw = pA.tile([P, G * P], BF, tag="pw")
                        nc.sync.dma_start(
                            pw[:], ins[f"pwem_{d}"][:, t0 * P:(t0 + G) * P])
                    nq = _ceil(nlive, QUAD)

                    def stage_a(q):
                        qt = q * QUAD
                        if l == 0:
                            return xq[:, qt * P:(qt + QUAD) * P]
                        # fused scale+replicate (all-SBUF DVE op),
                        # edge-major: repq[e, (t,d,j)] = atoms[e,t,j]*u^d
                        repq = pX.tile([P, QUAD * P], BF, tag="repq")
                        nc.vector.tensor_tensor(
                            out=repq[:].rearrange(
                                "p (t r j) -> p t r j", r=D, j=DIM),
                            in0=atoms[:, qt * DIM:(qt + QUAD) * DIM]
                            .rearrange("p (t o j) -> p t o j", o=1, j=DIM)
                            .to_broadcast((P, QUAD, D, DIM)),
                            in1=pw[:, qt * P:(qt + QUAD) * P]
                            .rearrange("p (t r j) -> p t r j", r=D, j=DIM),
                            op=ALU.mult)
                        tps = psT.tile([P, QUAD * P], BF, tag="tps")
                        for cq in range(QUAD):
                            nc.tensor.transpose(
                                out=tps[:, cq * P:(cq + 1) * P],
                                in_=repq[:, cq * P:(cq + 1) * P],
                                identity=ident_b[:])
                        xpt = pX.tile([P, QUAD * P], BF, tag="xpt")
                        nc.scalar.copy(out=xpt[:], in_=tps[:])
                        return xpt

                    def stage_b(q, xpt):
                        qt = q * QUAD
                        nonlocal yacc
                        oh4 = pO.tile([P, QUAD * P], BF, tag="oh4")
                        nc.vector.tensor_tensor(
                            out=oh4[:].rearrange("p (t n) -> p t n", n=P),
                            in0=iota4[:].rearrange("p (t n) -> p t n", n=P),
                            in1=dloc_sb[d][:, t0 + qt:t0 + qt + QUAD]
                            .rearrange("p (t o) -> p t o", o=1)
                            .to_broadcast((P, QUAD, P)),
                            op=ALU.is_equal)
                        mq = psM.tile([P, QUAD * DIM], FP, tag="mq")
                        for cq in range(QUAD):
                            t = t0 + qt + cq
                            # dead tiles/halves: xpt cols are zero, so a
                            # matmul with any wb writes zeros (keeps mq
                            # fully defined for the quad-wide silu)
                            sb0, sb1 = seg_bucket[t] if t < T_live else (0, 0)
                            b0, b1 = max(sb0, 0), max(sb1, 0)
                            col = xpt[:, cq * P:(cq + 1) * P]
                            if b0 == b1:
                                nc.tensor.matmul(
                                    out=mq[:, cq * DIM:(cq + 1) * DIM],
                                    lhsT=col, rhs=wb_ap(l, d, b0),
                                    start=True, stop=True)
                            else:
                                nc.tensor.matmul(
                                    out=mq[:HS, cq * DIM:(cq + 1) * DIM],
                                    lhsT=col[:, :HS],
                                    rhs=wb_ap(l, d, b0),
                                    start=True, stop=True,
                                    tile_position=(0, 0))
                                nc.tensor.matmul(
                                    out=mq[HS:, cq * DIM:(cq + 1) * DIM],
                                    lhsT=col[:, HS:],
                                    rhs=wb_ap(l, d, b1),
                                    start=True, stop=True,
                                    tile_position=(0, HS))
                        act4 = pX.tile([P, QUAD * DIM], BF, tag="act4")
                        nc.scalar.activation(
                            out=act4[:], in_=mq[:], func=AF.Silu)
                        # scatter the quad's tiles
                        for cq in range(QUAD):
                            t = t0 + qt + cq
                            if t >= T_live:
                                continue
                            kb = int(blk_of[t])
                            first = t == int(Toff[kb])
                            last = t == min(int(Toff[kb + 1]), T_live) - 1
                            if first:
                                yacc = psB.tile([DIM, P], FP, tag="yacc")
                            nc.tensor.matmul(
                                out=yacc[:],
                                lhsT=act4[:, cq * DIM:(cq + 1) * DIM],
                                rhs=oh4[:, cq * P:(cq + 1) * P],
                                start=first, stop=last)
                            if last:
                                n_hi = min((kb + 1) * P, npc[s])
                                seg = yT[s][:, kb * P:n_hi]
                                nc.vector.tensor_tensor(
                                    out=seg, in0=yacc[:, :n_hi - kb * P],
                                    in1=seg, op=ALU.add)

                    for q in range(nq):
                        stage_b(q, stage_a(q))

            # AllGather updated side for next layer
            if l < n_layers - 1:
                nblk = nblk_of[s]
                nfull = npc[s] // P
                with ExitStack() as gctx:
                    pg = gctx.enter_context(
                        tc.tile_pool(name=f"ag{l}{s}", bufs=2))
                    psG = gctx.enter_context(
                        tc.tile_pool(name=f"psG{l}{s}", bufs=4, space="PSUM"))
                    rows = pg.tile([P, nblk * DIM], BF, tag="agrows")
                    for kb in range(nblk):
                        tp = psG.tile([P, DIM], FP, tag="agT")
                        nc.tensor.transpose(
                            out=tp[:], in_=yT[s][:, kb * P:(kb + 1) * P],
                            identity=identity[:DIM, :DIM])
                        if kb % 2 == 0:
                            nc.vector.tensor_copy(
                                out=rows[:, kb * DIM:(kb + 1) * DIM], in_=tp[:])
                        else:
                            nc.scalar.copy(
                                out=rows[:, kb * DIM:(kb + 1) * DIM], in_=tp[:])
                    nc.sync.dma_start(
                        out=cc_in[s][:nfull * P, :]
                        .rearrange("(t p) k -> p t k", p=P),
                        in_=rows[:, :nfull * DIM]
                        .rearrange("p (t k) -> p t k", k=DIM))
                    rem = npc[s] - nfull * P
                    if rem:
                        nc.sync.dma_start(
                            out=cc_in[s][nfull * P:, :],
                            in_=rows[:rem, nfull * DIM:(nfull + 1) * DIM])
                    emit_collective(
                        "AllGather", ALU.bypass,
                        ins=[cc_in[s][:]],
                        outs=[y_tab[(s, l + 1)][:]],
                        replica_groups=replica_groups)

        for l in range(n_layers):
            dirs = (0, 1) if l % 2 == 0 else (1, 0)
            for d in dirs:
                emit_dirlayer(l, d)

        # ---------------- readout ----------------
        with ExitStack() as rctx:
            pr = rctx.enter_context(tc.tile_pool(name="ro", bufs=2))
            psR = rctx.enter_context(
                tc.tile_pool(name="psR", bufs=2, space="PSUM"))
            CH = 512
            n_chunks = sum(_ceil(npc[s], CH) for s in (0, 1))
            accs = pr.tile([1, max(n_chunks, 1)], FP)
            ci = 0
            for s in (0, 1):
                for c0 in range(0, npc[s], CH):
                    c1 = min(c0 + CH, npc[s])
                    dot_ps = psR.tile([1, CH], FP, tag="dot")
                    nc.tensor.matmul(
                        out=dot_ps[:, :c1 - c0], lhsT=ro_w_sb[:],
                        rhs=yT[s][:, c0:c1], start=True, stop=True)
                    sil = pr.tile([1, CH], FP, tag="sil")
                    nc.scalar.activation(
                        out=sil[:, :c1 - c0], in_=dot_ps[:, :c1 - c0],
                        func=AF.Silu, bias=float(meta["ro_b"]))
                    nc.vector.tensor_reduce(
                        out=accs[:, ci:ci + 1], in_=sil[:, :c1 - c0],
                        axis=mybir.AxisListType.X, op=ALU.add)
                    ci += 1
            total = pr.tile([1, 1], FP)
            nc.vector.tensor_reduce(
                out=total[:], in_=accs[:, :ci], axis=mybir.AxisListType.X,
                op=ALU.add)
            nc.sync.dma_start(out=ar_in[:], in_=total[:])
            emit_collective(
                "AllReduce", ALU.add,
                ins=[ar_in[:]], outs=[ar_out[:]],
                replica_groups=replica_groups)
            res = pr.tile([1, 1], FP)
            nc.sync.dma_start(out=res[:], in_=ar_out[:])
            nc.sync.dma_start(out=out_t[:], in_=res[:])

    nc.compile()
    return nc


# ======================== runner ========================
LAST_EXEC_NS = None
N_CORES = 8
GBATCH = 32


LAST_RES = None


def kernel(_trace=False, **inputs):
    global LAST_EXEC_NS, LAST_RES
    from concourse import bass_utils

    per_core, meta = prepare(inputs, n_cores=N_CORES, G=GBATCH)
    shapes = {k: v.shape for k, v in per_core[0].items()}
    nc = build_program(meta, shapes)
    in_maps = [{k: np.ascontiguousarray(v) for k, v in pc.items()}
               for pc in per_core]
    res = bass_utils.run_bass_kernel_spmd(
        nc, in_maps, core_ids=list(range(N_CORES)), trace=_trace)
    LAST_EXEC_NS = res.exec_time_ns
    LAST_RES = res
    return np.float32(res.results[0]["out"][0, 0])



# revision 44
# speedup vs baseline: 1.0378x; 1.0378x over previous
import sys
sys.path.insert(0, "/opt/trn_rl_repo")
"""Dimer GNN on 8 TRN2 cores — v4 (fused single-phase).

Edge-parallel sharding by owner core (dir 0: owner=dst updates y1, gathers
y0; dir 1: owner=src updates y0, gathers y1). ~50k edges per (core, dir).

Slot layout per direction (geometry COMMON across cores): tiles = columns
[P, T]; tiles are dst-block-pure (49 local blocks of 128 nodes); within a
block, edges are packed into 64-slot HALF-COLUMN strips that are bucket-pure
(strip count per (block, bucket) = max over cores -> common static geometry).
Tile t therefore has 1-2 static TP segments (partition ranges 0:64 / 64:128),
each with a static bucket.

Per direction-layer, per 4-tile quad (fused pipeline):
  atoms: l=0 from host-pregathered slab; l>=1 per-tile [128,1] indirect DMA
  row-gather from the AllGathered bf16 y_tab [N, 32].
  DVE rep-copy -> [128,(t,4rep,32j)]; 4 PE transposes -> psum atomsT_rep
  bf16 [128(d,j), 512]; DVE multiply with streamed pwrepT (u^d, channel-
  major, dead slots 0) -> xpT; per tile 1-2 MMs (lhsT=xpT segment, rhs=
  Wbflat[l,dir,bucket]) -> m [128e, 32k] f32 psum; ACT Silu -> act quad
  bf16; per tile: one-hot (DVE is_equal vs dloc) + scatter matmul
  accumulating into the block's [32,128] psum; block end: DVE add into
  channel-major yT [32, 6272] f32.
Layer end per side: PE transposes of yT -> bf16 rows -> AllGather y_tab.
Readout: chunked PE dot + ACT Silu + DVE reduce + AllReduce.
"""
import numpy as np

DIM = 32
STEP = 0.25   # gaussian sigma (fixed by the model)
N_MU = 21
N_SCAL = N_MU + 1
N_LAYERS = 3
NB = 10
BW = 5.0 / NB  # polynomial bucket width
P = 128
D = 4
HS = 64  # strip size (half column)


def poly_coeffs():
    mu = np.linspace(0.0, 5.0, N_MU)
    C = np.zeros((NB, D, N_SCAL))
    for b in range(NB):
        rr = np.linspace(b * BW, (b + 1) * BW, 257)
        u = (rr - b * BW) / BW * 2.0 - 1.0
        V = np.stack([u ** d for d in range(D)], 1)
        G = np.exp(-0.5 * ((rr[:, None] - mu[None, :]) / STEP) ** 2)
        G = np.concatenate([G, np.ones((len(rr), 1))], 1)
        C[b], *_ = np.linalg.lstsq(V, G, rcond=None)
    return C  # [NB, D, N_SCAL]


def _ceil(a, b):
    return -(-a // b)


def shard_direction(owner, other, r, n_cores, npc, G):
    """Fused layout. Returns per-core arrays + static geometry."""
    core_of = owner // npc
    bucket = np.clip((r / BW).astype(np.int64), 0, NB - 1)
    n_blocks = _ceil(npc, P)

    idx_by_core = [np.nonzero(core_of == c)[0] for c in range(n_cores)]
    loc_all = owner - core_of * npc
    blk_all = loc_all // P

    # common geometry: strips per (block, bucket) = max over cores
    nst = np.zeros((n_blocks, NB), np.int64)
    for c in range(n_cores):
        idx = idx_by_core[c]
        for kb in range(n_blocks):
            sel = idx[blk_all[idx] == kb]
            cnt = np.bincount(bucket[sel], minlength=NB)
            nst[kb] = np.maximum(nst[kb], _ceil(cnt, HS))
    S_kb = nst.sum(1)                       # strips per block
    tiles_kb = _ceil(S_kb, 2)               # 2 strips per tile
    Toff = np.concatenate([[0], np.cumsum(tiles_kb)]).astype(np.int64)
    T_live = int(Toff[-1])
    T = _ceil(T_live, G) * G

    # static segments: for each tile, list of (seg_lo_strip(0/1), bucket)
    # strip sigma of block kb: tile Toff[kb] + sigma//2, half sigma%2
    seg_bucket = np.full((T, 2), -1, np.int64)  # -1 = dead segment
    strip_pos = {}  # (kb, b) -> list of (tile, half) strips
    for kb in range(n_blocks):
        sigma = 0
        for b in range(NB):
            lst = []
            for _ in range(int(nst[kb, b])):
                t = int(Toff[kb] + sigma // 2)
                h = sigma % 2
                seg_bucket[t, h] = b
                lst.append((t, h))
                sigma += 1
            strip_pos[(kb, b)] = lst

    cores = []
    for c in range(n_cores):
        idx = idx_by_core[c]
        uA = np.zeros((P, T))
        vA = np.zeros((P, T))
        gidx = np.zeros((P, T), np.int32)
        dloc = np.full((P, T), 255.0)
        for kb in range(n_blocks):
            bidx = idx[blk_all[idx] == kb]
            bb = bucket[bidx]
            for b in range(NB):
                sel = bidx[bb == b]
                if len(sel) == 0:
                    continue
                sel = sel[np.argsort(other[sel], kind="stable")]
                strips = strip_pos[(kb, b)]
                for si, (t, h) in enumerate(strips):
                    seg = sel[si * HS:(si + 1) * HS]
                    if len(seg) == 0:
                        break
                    pr = h * HS + np.arange(len(seg))
                    uA[pr, t] = (r[seg] - b * BW) / BW * 2.0 - 1.0
                    vA[pr, t] = 1.0
                    gidx[pr, t] = other[seg]
                    dloc[pr, t] = (loc_all[seg] % P).astype(np.float64)
        cores.append(dict(uA=uA, vA=vA, gidx=gidx, dloc=dloc))

    meta = dict(T=T, T_live=T_live, Toff=Toff, seg_bucket=seg_bucket,
                n_blocks=n_blocks, tiles_kb=tiles_kb)
    return cores, meta


def prepare(inputs, n_cores=8, G=32):
    import ml_dtypes
    BF16 = ml_dtypes.bfloat16

    z0 = np.asarray(inputs["z0"]).astype(np.int64)
    z1 = np.asarray(inputs["z1"]).astype(np.int64)
    src = np.asarray(inputs["src"]).astype(np.int64)
    dst = np.asarray(inputs["dst"]).astype(np.int64)
    r = np.asarray(inputs["r"], np.float64)
    emb_w = np.asarray(inputs["emb_w"], np.float32)
    emb_b = np.asarray(inputs["emb_b"], np.float32)
    w_s2d = np.asarray(inputs["w_s2d"], np.float32)
    w_d2s = np.asarray(inputs["w_d2s"], np.float32)
    ro_w = np.asarray(inputs["ro_w"], np.float32)
    ro_b = np.asarray(inputs["ro_b"], np.float32)
    N0, N1 = len(z0), len(z1)
    npc0, npc1 = N0 // n_cores, N1 // n_cores

    C = poly_coeffs()
    scale = 1.0 / np.sqrt(N_SCAL * DIM) / np.sqrt(N0 + N1)
    wb = np.zeros((N_LAYERS, 2, NB, D * DIM, DIM), np.float32)
    for l in range(N_LAYERS):
        wb[l, 0] = np.einsum("bdi,ijk->bdjk", C, w_s2d[l]).reshape(NB, D * DIM, DIM) * scale
        wb[l, 1] = np.einsum("bdi,ijk->bdjk", C, w_d2s[l]).reshape(NB, D * DIM, DIM) * scale
    wb_sb = np.ascontiguousarray(
        wb.transpose(3, 0, 1, 2, 4).reshape(D * DIM, N_LAYERS * 2 * NB * DIM)
    ).astype(BF16)

    emb_eff = (emb_w + emb_b[None, :]).astype(np.float32)
    emb_b16 = emb_eff.astype(BF16)

    s2d_cores, s2d_meta = shard_direction(dst, src, r, n_cores, npc1, G)
    d2s_cores, d2s_meta = shard_direction(src, dst, r, n_cores, npc0, G)
    metas = {0: s2d_meta, 1: d2s_meta}
    z_of_side = {0: z0, 1: z1}
    gside_of_dir = {0: 0, 1: 1}
    npc_of_side = {0: npc0, 1: npc1}

    per_core = []
    for c in range(n_cores):
        dd = {}
        for d, cores in ((0, s2d_cores), (1, d2s_cores)):
            cd = cores[c]
            T = metas[d]["T"]
            u, v = cd["uA"], cd["vA"]
            pows = np.stack([v * (u ** dg) for dg in range(D)], 0)
            pw = np.repeat(pows, DIM, axis=0)  # [128 (d,j), P e, T]
            # edge-major: [P e, T*(d,j)] for the fused scale-replicate copy
            pw_em = np.ascontiguousarray(
                pw.transpose(1, 2, 0).reshape(P, T * D * DIM)).astype(BF16)
            zg = z_of_side[gside_of_dir[d]]
            # l=0 slab: pre-transposed, pre-scaled lhsT  [128 (d,j), T*P]
            atT = emb_eff[zg[cd["gidx"]]]            # [P e, T, DIM j] f32
            atT = atT.transpose(2, 1, 0)             # [DIM j, T, P e]
            atT_rep = np.tile(atT, (D, 1, 1)).reshape(D * DIM, T * P)
            pw_t = pw.transpose(0, 2, 1).reshape(D * DIM, T * P)
            sfx = f"_{d}"
            dd["pwem" + sfx] = pw_em
            dd["xpT0" + sfx] = (pw_t * atT_rep).astype(BF16)
            dd["gidx" + sfx] = np.ascontiguousarray(cd["gidx"])
            dd["dloc" + sfx] = np.ascontiguousarray(cd["dloc"].astype(BF16))
        for s in (0, 1):
            npc = npc_of_side[s]
            nblk = _ceil(npc, P)
            zz = np.zeros(nblk * P, np.int64)
            zz[:npc] = z_of_side[s][c * npc:(c + 1) * npc]
            yT0 = emb_eff[zz].T.copy()
            yT0[:, npc:] = 0.0
            dd[f"yT0_{s}"] = np.ascontiguousarray(yT0)  # [32, nblk*P] f32
            dd[f"ytab0_{s}"] = emb_b16[z_of_side[s]]     # [N, 32] bf16
        dd["wb_sb"] = wb_sb
        dd["ro_w"] = ro_w
        per_core.append(dd)

    meta = dict(m0=s2d_meta, m1=d2s_meta, n_cores=n_cores, G=G,
                npc0=npc0, npc1=npc1, ro_b=float(ro_b[0]),
                N_of_side={0: N0, 1: N1})
    return per_core, meta


# ======================== bass program ========================
from contextlib import ExitStack

from concourse import bass, mybir
from concourse import bacc
import concourse.tile as tile
from concourse.masks import make_identity

FP = mybir.dt.float32
BF = mybir.dt.bfloat16
I32 = mybir.dt.int32
AF = mybir.ActivationFunctionType
ALU = mybir.AluOpType
QUAD = 8


def build_program(meta, shapes, n_layers=3, sim_mode=False):
    n_cores = meta["n_cores"]
    G = meta["G"]
    npc = {0: meta["npc0"], 1: meta["npc1"]}
    N_side = meta["N_of_side"]
    metas = {0: meta["m0"], 1: meta["m1"]}
    side_of_dir = {0: 1, 1: 0}
    gside_of_dir = {0: 0, 1: 1}

    nc = bacc.Bacc("TRN2", target_bir_lowering=False, debug=False,
                   num_devices=1 if sim_mode else n_cores)

    def emit_collective(kind, op, ins, outs, replica_groups):
        if sim_mode:
            # single-core timing model: stand in a local DMA of the same
            # output footprint (collective itself modeled separately)
            nc.gpsimd.dma_start(out=outs[0], in_=outs[0])
        else:
            nc.gpsimd.collective_compute(
                kind, op, ins=ins, outs=outs, replica_groups=replica_groups)

    ins = {}
    for name, shp in shapes.items():
        if name.startswith("gidx"):
            dt = I32
        elif name.startswith(("yT0", "ro_w")):
            dt = FP
        else:
            dt = BF
        ins[name] = nc.dram_tensor(name, list(shp), dt, kind="ExternalInput")

    out_t = nc.dram_tensor("out", [1, 1], FP, kind="ExternalOutput")

    cc_in = {s: nc.dram_tensor(f"cc_in_{s}", [npc[s], DIM], BF)
             for s in (0, 1)}
    y_tab = {}
    for s in (0, 1):
        for l in (1, 2):
            y_tab[(s, l)] = nc.dram_tensor(
                f"y_tab_{s}_{l}", [N_side[s], DIM], BF, addr_space="Shared")
    ar_in = nc.dram_tensor("ar_in", [1, 1], FP)
    ar_out = nc.dram_tensor("ar_out", [1, 1], FP, addr_space="Shared")

    replica_groups = [list(range(n_cores))]
    WBCOLS = n_layers * 2 * NB * DIM
    nblk_of = {s: _ceil(npc[s], P) for s in (0, 1)}

    with tile.TileContext(nc) as tc, ExitStack() as ctx:
        const = ctx.enter_context(tc.tile_pool(name="const", bufs=1))
        identity = const.tile([P, P], FP)
        make_identity(nc, identity[:])
        ident_b = const.tile([P, P], BF)
        nc.vector.tensor_copy(out=ident_b[:], in_=identity[:])
        iota_i = const.tile([P, P], I32)
        nc.gpsimd.iota(iota_i[:], [[1, P]], base=0, channel_multiplier=0)
        iota_b = const.tile([P, P], BF)
        nc.vector.tensor_copy(out=iota_b[:], in_=iota_i[:])
        iota4 = const.tile([P, GBATCH * P], BF)
        for _q in range(GBATCH):
            nc.vector.tensor_copy(out=iota4[:, _q * P:(_q + 1) * P],
                                  in_=iota_b[:])
        ro_w_sb = const.tile([DIM, 1], FP)
        nc.sync.dma_start(ro_w_sb[:], ins["ro_w"][:])
        wb_sb = const.tile([P, WBCOLS], BF)
        nc.sync.dma_start(wb_sb[:], ins["wb_sb"][:])

        def wb_ap(l, d, b):
            off = ((l * 2 + d) * NB + b) * DIM
            return wb_sb[:, off:off + DIM]

        emeta = ctx.enter_context(tc.tile_pool(name="emeta", bufs=1))
        gidx_sb, dloc_sb = {}, {}
        for d in (0, 1):
            T = metas[d]["T"]
            gidx_sb[d] = emeta.tile([P, T], I32, name=f"gidx{d}", tag=f"gidx{d}")
            nc.sync.dma_start(gidx_sb[d][:], ins[f"gidx_{d}"][:])
            dloc_sb[d] = emeta.tile([P, T], BF, name=f"dloc{d}", tag=f"dloc{d}")
            nc.sync.dma_start(dloc_sb[d][:], ins[f"dloc_{d}"][:])

        yT = {}
        for s in (0, 1):
            yT[s] = const.tile([DIM, nblk_of[s] * P], FP, name=f"yT{s}", tag=f"yT{s}")
            nc.sync.dma_start(yT[s][:], ins[f"yT0_{s}"][:])

        # ---------------- fused direction-layer ----------------
        def emit_dirlayer(l, d):
            m = metas[d]
            T, T_live = m["T"], m["T_live"]
            Toff = m["Toff"]
            seg_bucket = m["seg_bucket"]
            n_blocks = m["n_blocks"]
            s = side_of_dir[d]
            gs = gside_of_dir[d]
            # block of tile, and first/last live tile per block
            blk_of = np.searchsorted(Toff, np.arange(T), side="right") - 1
            with ExitStack() as actx:
                pA = actx.enter_context(tc.tile_pool(name=f"pA{l}{d}", bufs=3))
                pX = actx.enter_context(tc.tile_pool(name=f"pX{l}{d}", bufs=4))
                pO = actx.enter_context(tc.tile_pool(name=f"pO{l}{d}", bufs=2))
                psT = actx.enter_context(
                    tc.tile_pool(name=f"psT{l}{d}", bufs=3, space="PSUM"))
                psM = actx.enter_context(
                    tc.tile_pool(name=f"psM{l}{d}", bufs=3, space="PSUM"))
                psB = actx.enter_context(
                    tc.tile_pool(name=f"psB{l}{d}", bufs=2, space="PSUM"))
                yacc = None
                nG = T // G
                for g in range(nG):
                    t0 = g * G
                    nlive = max(0, min(G, T_live - t0))
                    if nlive == 0:
                        continue
                    if l == 0:
                        xq = pA.tile([P, G * P], BF, tag="pw")
                        nc.sync.dma_start(
                            xq[:], ins[f"xpT0_{d}"][:, t0 * P:(t0 + G) * P])
                    else:
                        atoms = pA.tile([P, G * DIM], BF, tag="atoms")
                        tab = y_tab[(gs, l)]
                        nc.gpsimd.indirect_dma_start(
                            out=atoms[:].rearrange(
                                "p (t j) -> p t j", j=DIM),
                            out_offset=None, in_=tab[:],
                            in_offset=bass.IndirectOffsetOnAxis(
                                ap=gidx_sb[d][:, t0:t0 + G], axis=0))
                        pw = pA.tile([P, G * P], BF, tag="pw")
                        nc.sync.dma_start(
                            pw[:], ins[f"pwem_{d}"][:, t0 * P:(t0 + G) * P])
                    nq = _ceil(nlive, QUAD)
                    # group-wide one-hot build: one DVE op for all G tiles
                    ohg = pO.tile([P, G * P], BF, tag="oh4")
                    nc.vector.tensor_tensor(
                        out=ohg[:].rearrange("p (t n) -> p t n", n=P),
                        in0=iota4[:].rearrange("p (t n) -> p t n", n=P),
                        in1=dloc_sb[d][:, t0:t0 + G]
                        .rearrange("p (t o) -> p t o", o=1)
                        .to_broadcast((P, G, P)),
                        op=ALU.is_equal)

                    def stage_a(q):
                        qt = q * QUAD
                        if l == 0:
                            return xq[:, qt * P:(qt + QUAD) * P]
                        # fused scale+replicate (all-SBUF DVE op),
                        # edge-major: repq[e, (t,d,j)] = atoms[e,t,j]*u^d
                        repq = pX.tile([P, QUAD * P], BF, tag="repq")
                        nc.vector.tensor_tensor(
                            out=repq[:].rearrange(
                                "p (t r j) -> p t r j", r=D, j=DIM),
                            in0=atoms[:, qt * DIM:(qt + QUAD) * DIM]
                            .rearrange("p (t o j) -> p t o j", o=1, j=DIM)
                            .to_broadcast((P, QUAD, D, DIM)),
                            in1=pw[:, qt * P:(qt + QUAD) * P]
                            .rearrange("p (t r j) -> p t r j", r=D, j=DIM),
                            op=ALU.mult)
                        tps = psT.tile([P, QUAD * P], BF, tag="tps")
                        for cq in range(QUAD):
                            nc.tensor.transpose(
                                out=tps[:, cq * P:(cq + 1) * P],
                                in_=repq[:, cq * P:(cq + 1) * P],
                                identity=ident_b[:])
                        xpt = pX.tile([P, QUAD * P], BF, tag="xpt")
                        nc.scalar.copy(out=xpt[:], in_=tps[:])
                        return xpt

                    def stage_b(q, xpt):
                        qt = q * QUAD
                        nonlocal yacc
                        oh4 = ohg[:, qt * P:(qt + QUAD) * P]
                        mq = psM.tile([P, QUAD * DIM], FP, tag="mq")
                        for cq in range(QUAD):
                            t = t0 + qt + cq
                            # dead tiles/halves: xpt cols are zero, so a
                            # matmul with any wb writes zeros (keeps mq
                            # fully defined for the quad-wide silu)
                            sb0, sb1 = seg_bucket[t] if t < T_live else (0, 0)
                            b0, b1 = max(sb0, 0), max(sb1, 0)
                            col = xpt[:, cq * P:(cq + 1) * P]
                            if b0 == b1:
                                nc.tensor.matmul(
                                    out=mq[:, cq * DIM:(cq + 1) * DIM],
                                    lhsT=col, rhs=wb_ap(l, d, b0),
                                    start=True, stop=True)
                            else:
                                nc.tensor.matmul(
                                    out=mq[:HS, cq * DIM:(cq + 1) * DIM],
                                    lhsT=col[:, :HS],
                                    rhs=wb_ap(l, d, b0),
                                    start=True, stop=True,
                                    tile_position=(0, 0))
                                nc.tensor.matmul(
                                    out=mq[HS:, cq * DIM:(cq + 1) * DIM],
                                    lhsT=col[:, HS:],
                                    rhs=wb_ap(l, d, b1),
                                    start=True, stop=True,
                                    tile_position=(0, HS))
                        act4 = pX.tile([P, QUAD * DIM], BF, tag="act4")
                        nc.scalar.activation(
                            out=act4[:], in_=mq[:], func=AF.Silu)
                        # scatter the quad's tiles
                        for cq in range(QUAD):
                            t = t0 + qt + cq
                            if t >= T_live:
                                continue
                            kb = int(blk_of[t])
                            first = t == int(Toff[kb])
                            last = t == min(int(Toff[kb + 1]), T_live) - 1
                            if first:
                                yacc = psB.tile([DIM, P], FP, tag="yacc")
                            nc.tensor.matmul(
                                out=yacc[:],
                                lhsT=act4[:, cq * DIM:(cq + 1) * DIM],
                                rhs=oh4[:, cq * P:(cq + 1) * P],
                                start=first, stop=last)
                            if last:
                                n_hi = min((kb + 1) * P, npc[s])
                                seg = yT[s][:, kb * P:n_hi]
                                nc.vector.tensor_tensor(
                                    out=seg, in0=yacc[:, :n_hi - kb * P],
                                    in1=seg, op=ALU.add)

                    for q in range(nq):
                        stage_b(q, stage_a(q))

            # AllGather updated side for next layer
            if l < n_layers - 1:
                nblk = nblk_of[s]
                nfull = npc[s] // P
                with ExitStack() as gctx:
                    pg = gctx.enter_context(
                        tc.tile_pool(name=f"ag{l}{s}", bufs=2))
                    psG = gctx.enter_context(
                        tc.tile_pool(name=f"psG{l}{s}", bufs=4, space="PSUM"))
                    rows = pg.tile([P, nblk * DIM], BF, tag="agrows")
                    for kb in range(nblk):
                        tp = psG.tile([P, DIM], FP, tag="agT")
                        nc.tensor.transpose(
                            out=tp[:], in_=yT[s][:, kb * P:(kb + 1) * P],
                            identity=identity[:DIM, :DIM])
                        if kb % 2 == 0:
                            nc.vector.tensor_copy(
                                out=rows[:, kb * DIM:(kb + 1) * DIM], in_=tp[:])
                        else:
                            nc.scalar.copy(
                                out=rows[:, kb * DIM:(kb + 1) * DIM], in_=tp[:])
                    nc.sync.dma_start(
                        out=cc_in[s][:nfull * P, :]
                        .rearrange("(t p) k -> p t k", p=P),
                        in_=rows[:, :nfull * DIM]
                        .rearrange("p (t k) -> p t k", k=DIM))
                    rem = npc[s] - nfull * P
                    if rem:
                        nc.sync.dma_start(
                            out=cc_in[s][nfull * P:, :],
                            in_=rows[:rem, nfull * DIM:(nfull + 1) * DIM])
                    emit_collective(
                        "AllGather", ALU.bypass,
                        ins=[cc_in[s][:]],
                        outs=[y_tab[(s, l + 1)][:]],
                        replica_groups=replica_groups)

        for l in range(n_layers):
            dirs = (0, 1) if l % 2 == 0 else (1, 0)
            for d in dirs:
                emit_dirlayer(l, d)

        # ---------------- readout ----------------
        with ExitStack() as rctx:
            pr = rctx.enter_context(tc.tile_pool(name="ro", bufs=2))
            psR = rctx.enter_context(
                tc.tile_pool(name="psR", bufs=2, space="PSUM"))
            CH = 512
            n_chunks = sum(_ceil(npc[s], CH) for s in (0, 1))
            accs = pr.tile([1, max(n_chunks, 1)], FP)
            ci = 0
            for s in (0, 1):
                for c0 in range(0, npc[s], CH):
                    c1 = min(c0 + CH, npc[s])
                    dot_ps = psR.tile([1, CH], FP, tag="dot")
                    nc.tensor.matmul(
                        out=dot_ps[:, :c1 - c0], lhsT=ro_w_sb[:],
                        rhs=yT[s][:, c0:c1], start=True, stop=True)
                    sil = pr.tile([1, CH], FP, tag="sil")
                    nc.scalar.activation(
                        out=sil[:, :c1 - c0], in_=dot_ps[:, :c1 - c0],
                        func=AF.Silu, bias=float(meta["ro_b"]))
                    nc.vector.tensor_reduce(
                        out=accs[:, ci:ci + 1], in_=sil[:, :c1 - c0],
                        axis=mybir.AxisListType.X, op=ALU.add)
                    ci += 1
            total = pr.tile([1, 1], FP)
            nc.vector.tensor_reduce(
                out=total[:], in_=accs[:, :ci], axis=mybir.AxisListType.X,
                op=ALU.add)
            nc.sync.dma_start(out=ar_in[:], in_=total[:])
            emit_collective(
                "AllReduce", ALU.add,
                ins=[ar_in[:]], outs=[ar_out[:]],
                replica_groups=replica_groups)
            res = pr.tile([1, 1], FP)
            nc.sync.dma_start(out=res[:], in_=ar_out[:])
            nc.sync.dma_start(out=out_t[:], in_=res[:])

    nc.compile()
    return nc


# ======================== runner ========================
LAST_EXEC_NS = None
N_CORES = 8
GBATCH = 32


LAST_RES = None


def kernel(_trace=False, **inputs):
    global LAST_EXEC_NS, LAST_RES
    from concourse import bass_utils

    per_core, meta = prepare(inputs, n_cores=N_CORES, G=GBATCH)
    shapes = {k: v.shape for k, v in per_core[0].items()}
    nc = build_program(meta, shapes)
    in_maps = [{k: np.ascontiguousarray(v) for k, v in pc.items()}
               for pc in per_core]
    res = bass_utils.run_bass_kernel_spmd(
        nc, in_maps, core_ids=list(range(N_CORES)), trace=_trace)
    LAST_EXEC_NS = res.exec_time_ns
    LAST_RES = res
    return np.float32(res.results[0]["out"][0, 0])

